# revision 1
# baseline (speedup 1.0000x reference)
"""GroupGMM Trainium2 kernel.

Computes, for B=8192 samples with soft group-mixture weights over G=32 groups:
    logits = einsum("bi,gio,bg->bo", x, W_pi, g) + g @ b_pi        [B, 16]
    loc    = einsum(... W_mu ...)   + g @ b_mu                     [B, 512]
    scale  = softplus(einsum(... W_sigma ...) + g @ b_sigma)+1e-7  [B, 512]
    out    = concat([logits, loc, scale], -1)                      [B, 1040]

Strategy: data-parallel over batch across 8 NeuronCores (1024 rows each).
The group einsum is folded into one matmul with contraction K = G*I = 16384
via z[b,(g,i)] = g[b,g] * x[b,i]. Per 128-sample chunk, z^T K-tiles are
built on the Vector engine (x^T tile * broadcast gate row, both bf16,
host-pre-transposed/broadcast), and the PE accumulates all 128 K-tiles
into PSUM. PSUM capacity (8 banks) fits mu+sigma accumulators for 3
sample-chunks, so the batch is processed in 3 sweeps ([0..2],[3..5],[6..7])
with the weight K-tiles re-streamed from HBM per sweep on the sync HWDGE
queue (all other traffic uses the gpsimd queue so the W stream is never
blocked). The bias term g @ b_cat is precomputed on the host and added at
drain time on DVE; sigma gets softplus via ACT Exp+Ln (one shared table).
"""

import numpy as np
import ml_dtypes

import concourse.bass as bass
import concourse.tile as tile
from concourse import bacc, mybir
from concourse.bass_utils import run_bass_kernel_spmd

B, I, G, C, D = 8192, 512, 32, 16, 32
CD = C * D                      # 512
OUT_W = C + 2 * CD              # 1040
NCORES = 8
BLOC = B // NCORES              # 1024
KTOT = G * I                    # 16384
NKT = KTOT // 128               # 128 K-tiles
NMC = BLOC // 128               # 8 sample chunks per core
SWEEPS = [[0, 1, 2], [3, 4, 5], [6, 7]]

BF16 = mybir.dt.bfloat16
F32 = mybir.dt.float32

_cache: dict = {}


def _build_program():
    if "nc" in _cache:
        return _cache["nc"]
    from contextlib import ExitStack

    nc = bacc.Bacc("TRN2", target_bir_lowering=False, debug=False)

    xt_d = nc.dram_tensor("xt", [I, BLOC], BF16, kind="ExternalInput")
    gb_d = nc.dram_tensor("gb", [G, 128, BLOC], BF16, kind="ExternalInput")
    w_d = nc.dram_tensor("w", [NKT, 128, OUT_W], BF16, kind="ExternalInput")
    bias_d = nc.dram_tensor("bias", [BLOC, OUT_W], F32, kind="ExternalInput")
    out_d = nc.dram_tensor("out", [BLOC, OUT_W], F32, kind="ExternalOutput")

    with tile.TileContext(nc) as tc, ExitStack() as ctx:
        res = ctx.enter_context(tc.tile_pool(name="res", bufs=1))
        wp = ctx.enter_context(tc.tile_pool(name="wp", bufs=6))
        zp = ctx.enter_context(tc.tile_pool(name="zp", bufs=8))
        op = ctx.enter_context(tc.tile_pool(name="op", bufs=3))
        bp = ctx.enter_context(tc.tile_pool(name="bp", bufs=4))
        pp = ctx.enter_context(tc.tile_pool(name="pp", bufs=1, space="PSUM"))

        # Startup-critical loads go on the sync HWDGE queue ahead of the W
        # stream: the first gate tile and x^T block 0; x^T blocks 1-3 are
        # interleaved with the first W tiles so the opening matmul group
        # never waits behind a megabyte of resident loads.
        gb_t = [None] * G
        gb_t[0] = res.tile([128, BLOC], BF16, name="gbt0", tag="gbt0")
        nc.sync.dma_start(gb_t[0][:], gb_d[0])
        xt_t = []
        for ib in range(I // 128):
            t = res.tile([128, BLOC], BF16, name=f"xtt{ib}", tag=f"xtt{ib}")
            xt_t.append(t)
        nc.sync.dma_start(xt_t[0][:], xt_d[0:128, :])

        carry_z: dict = {}

        def gen_z(s, kt, mcs):
            gi = kt // 4
            ib = kt % 4
            m0 = mcs[0] * 128
            mw = len(mcs) * 128
            zt = zp.tile([128, mw], BF16, name=f"zt{s}_{kt}", tag="zt")
            nc.vector.tensor_mul(zt[:], xt_t[ib][:, m0:m0 + mw],
                                 gb_t[gi][:, m0:m0 + mw])
            return zt

        for s, mcs in enumerate(SWEEPS):
            # ppi gets the 8th PSUM bank as a second slot so the next sweep
            # never waits on this sweep's pi drain.
            ppi = pp.tile([128, 16 * len(mcs)], F32, name=f"ppi{s}",
                          tag="ppi", bufs=2)
            pmu, psg, bt = {}, {}, {}
            for j, mc in enumerate(mcs):
                pmu[mc] = pp.tile([128, CD], F32, name=f"pmu{s}_{j}",
                                  tag="pmu", bufs=3)
                psg[mc] = pp.tile([128, CD], F32, name=f"psg{s}_{j}",
                                  tag="psg", bufs=3)

            for kt in range(NKT):
                gi = kt // 4
                ib = kt % 4
                if s == 0 and 1 <= kt <= 3:
                    nc.sync.dma_start(xt_t[kt][:],
                                      xt_d[kt * 128:(kt + 1) * 128, :])
                if s == 0 and ib == 0 and gi + 1 < G:
                    # Load gate tiles lazily on the fast queue, paced one
                    # group ahead of use, so PE isn't stuck behind 8.4MB of
                    # resident loads at startup.
                    t = res.tile([128, BLOC], BF16, name=f"gbt{gi + 1}",
                                 tag=f"gbt{gi + 1}")
                    nc.sync.dma_start(t[:], gb_d[gi + 1])
                    gb_t[gi + 1] = t
                if kt == 16:
                    # Bias tiles for this sweep's drain. On the sync queue
                    # mid-sweep: HWDGE executes in order, so they can't jump
                    # ahead of startup-critical loads on the shared DMA
                    # engines (gpsimd would issue them immediately).
                    for j2, mc2 in enumerate(mcs):
                        bt[mc2] = bp.tile([128, OUT_W], F32,
                                          name=f"bt{s}_{j2}", tag="bt")
                        nc.sync.dma_start(
                            bt[mc2][:],
                            bias_d[mc2 * 128:(mc2 + 1) * 128, :])
                wt = wp.tile([128, OUT_W], BF16, name=f"wt{s}_{kt}", tag="wt")
                nc.sync.dma_start(wt[:], w_d[kt])
                zt = carry_z.pop((s, kt), None)
                if zt is None:
                    zt = gen_z(s, kt, mcs)
                first = kt == 0
                last = kt == NKT - 1
                if last:
                    # Final K-tile: run the sigma matmuls first so psg is
                    # ready earliest — its drain (add→Exp→Ln) is the long
                    # serial chain at the end of the sweep.
                    for j, mc in enumerate(mcs):
                        lhs = zt[:, j * 128:(j + 1) * 128]
                        nc.tensor.matmul(psg[mc][:], lhs, wt[:, C + CD:],
                                         start=False, stop=True)
                    for j, mc in enumerate(mcs):
                        lhs = zt[:, j * 128:(j + 1) * 128]
                        nc.tensor.matmul(pmu[mc][:], lhs, wt[:, C:C + CD],
                                         start=False, stop=True)
                        nc.tensor.matmul(ppi[:, j * 16:(j + 1) * 16], lhs,
                                         wt[:, 0:C], start=False, stop=True,
                                         skip_group_check=True)
                    continue
                for j, mc in enumerate(mcs):
                    lhs = zt[:, j * 128:(j + 1) * 128]
                    nc.tensor.matmul(pmu[mc][:], lhs, wt[:, C:C + CD],
                                     start=first, stop=False)
                    nc.tensor.matmul(psg[mc][:], lhs, wt[:, C + CD:],
                                     start=first, stop=False)
                    # start=True marks the whole 2KB bank pending-zero, so
                    # only the first matmul into the shared pi bank sets it;
                    # later slices' first writes overwrite via pending-zero.
                    nc.tensor.matmul(ppi[:, j * 16:(j + 1) * 16], lhs,
                                     wt[:, 0:C], start=(first and j == 0),
                                     stop=False, skip_group_check=True)

            # Queue the next sweep's first z-tiles on DVE ahead of the drain
            # work so PE can restart immediately at the sweep boundary.
            if s + 1 < len(SWEEPS):
                for kt in range(3):
                    carry_z[(s + 1, kt)] = gen_z(s + 1, kt, SWEEPS[s + 1])

            # Drain, phase-batched so ACT runs exp,exp,..,ln,ln,.. — the
            # act-table chooser puts Exp and Ln in different function sets,
            # and interleaving them costs a 1.3us table reload per call.
            # softplus(v) = ln(exp(v) + 1); the reference's +1e-7 is dropped
            # (5e-7 relative effect, far below bf16 noise).
            ots, ets = {}, {}
            for j, mc in enumerate(mcs):
                # mu-add first frees this pmu slot for the next sweep's
                # opening matmul; ei-add right after feeds ACT and frees psg.
                ot = op.tile([128, OUT_W], F32, name=f"ot{s}_{j}", tag="ot")
                nc.vector.tensor_add(ot[:, C:C + CD], pmu[mc][:],
                                     bt[mc][:, C:C + CD])
                ei = op.tile([128, CD], F32, name=f"ei{s}_{j}", tag="ei",
                             bufs=3)
                nc.vector.tensor_add(ei[:], psg[mc][:], bt[mc][:, C + CD:])
                ots[mc] = ot
                ets[mc] = ei
            for j, mc in enumerate(mcs):
                et = op.tile([128, CD], F32, name=f"et{s}_{j}", tag="et",
                             bufs=3)
                nc.scalar.activation(et[:], ets[mc][:],
                                     mybir.ActivationFunctionType.Exp)
                ets[mc] = et
            for j, mc in enumerate(mcs):
                ot = ots[mc]
                nc.vector.tensor_add(ot[:, 0:C], ppi[:, j * 16:(j + 1) * 16],
                                     bt[mc][:, 0:C])
                nc.gpsimd.dma_start(out_d[mc * 128:(mc + 1) * 128, 0:C + CD],
                                    ot[:, 0:C + CD])
            for j, mc in enumerate(mcs):
                ot = ots[mc]
                nc.scalar.activation(ot[:, C + CD:], ets[mc][:],
                                     mybir.ActivationFunctionType.Ln,
                                     bias=1.0)
                nc.gpsimd.dma_start(out_d[mc * 128:(mc + 1) * 128, C + CD:],
                                    ot[:, C + CD:])

    nc.compile()
    _cache["nc"] = nc
    return nc


def _prep_shared(W_mu, b_mu, W_sigma, b_sigma, W_pi, b_pi):
    bf16 = ml_dtypes.bfloat16
    # Column order matches the reference output: [logits | loc | scale].
    w_cat = np.concatenate([W_pi, W_mu, W_sigma], axis=-1)      # [G, I, 1040]
    w_np = np.ascontiguousarray(
        w_cat.reshape(NKT, 128, OUT_W).astype(bf16))
    b_cat = np.concatenate([b_pi, b_mu, b_sigma],
                           axis=-1).astype(np.float32)          # [G, 1040]
    return w_np, b_cat


def _core_inputs(x, g, w_np, b_cat, c):
    bf16 = ml_dtypes.bfloat16
    xs = x[c * BLOC:(c + 1) * BLOC]
    gs = g[c * BLOC:(c + 1) * BLOC]
    xT = np.ascontiguousarray(xs.T.astype(bf16))                # [512, 1024]
    gT = gs.T.astype(bf16)                                      # [32, 1024]
    gb = np.ascontiguousarray(
        np.broadcast_to(gT[:, None, :], (G, 128, BLOC)))        # [32,128,1024]
    bias = np.ascontiguousarray(gs.astype(np.float32) @ b_cat)  # [1024, 1040]
    return {"xt": xT, "gb": gb, "w": w_np, "bias": bias}


def kernel(x, g, W_mu, b_mu, W_sigma, b_sigma, W_pi, b_pi):
    nc = _build_program()
    w_np, b_cat = _prep_shared(W_mu, b_mu, W_sigma, b_sigma, W_pi, b_pi)
    in_maps = [_core_inputs(x, g, w_np, b_cat, c) for c in range(NCORES)]
    res = run_bass_kernel_spmd(nc, in_maps, core_ids=list(range(NCORES)))
    out = np.concatenate(
        [res.results[c]["out"] for c in range(NCORES)], axis=0)
    return np.ascontiguousarray(out.astype(np.float32))



# revision 3
# speedup vs baseline: 1.9467x; 1.9467x over previous
"""GroupGMM Trainium2 kernel (fp8 DoubleRow).

Computes, for B=8192 samples with soft group-mixture weights over G=32 groups:
    logits = einsum("bi,gio,bg->bo", x, W_pi, g) + g @ b_pi        [B, 16]
    loc    = einsum(... W_mu ...)   + g @ b_mu                     [B, 512]
    scale  = softplus(einsum(... W_sigma ...) + g @ b_sigma)+1e-7  [B, 512]
    out    = concat([logits, loc, scale], -1)                      [B, 1040]

Strategy: data-parallel over batch across 8 NeuronCores (1024 rows each).
The group einsum is folded into one matmul with contraction K = G*I = 16384
via z[b,(g,i)] = g[b,g] * x[b,i], run in fp8e4 (e4m3) with the PE's
DoubleRow perf mode (two 128-row K-tiles per pass at 0.5 cycles/row;
measured overall rel err ~1e-2 vs the 2e-2 gate). K-tiles go in 64 pairs:
per pair a [128, 2, mw] bf16 z tile is built with ONE DVE multiply (x^T
pair slice * gate row broadcast via a stride-0 AP, 2x DVE mode), then cast
bf16->fp8 on a rotating engine (ACT/Pool/DVE) because a DVE multiply
cannot write fp8 at 2x but a copy can, and at fp8-DR speed the PE outpaces
any single helper engine. Per 128-sample chunk each pair issues 3 DR
matmuls (mu 512, sigma 512, pi 16 cols; lhsT [128, 2, 128]) — the exact
shapes concourse's tile_matmul emits, so walrus ISA checks pass. PSUM:
3 chunks x (mu+sg) + 2 pi banks = 8, so the batch runs in 3 sweeps
([0..2],[3..5],[6..7]). The fp8 W pair tiles (266KB each) are re-streamed
per sweep, split across the sync and gpsimd HWDGE queues (one queue alone
cannot match the fp8 PE pace); gate tiles load as per-sweep column slices
alternating between the queues. The bias term g @ b_cat is precomputed on
the host in f32 and added at drain time on DVE; sigma gets softplus via
ACT Exp+Ln.
"""

import numpy as np
import ml_dtypes

import concourse.bass as bass
import concourse.tile as tile
from concourse import bacc, mybir
from concourse.bass_utils import run_bass_kernel_spmd

B, I, G, C, D = 8192, 512, 32, 16, 32
CD = C * D                      # 512
OUT_W = C + 2 * CD              # 1040
NCORES = 8
BLOC = B // NCORES              # 1024
KTOT = G * I                    # 16384
NKT = KTOT // 128               # 128 K-tiles
NPAIR = NKT // 2                # 64 DoubleRow K-tile pairs
NMC = BLOC // 128               # 8 sample chunks per core
SWEEPS = [[0, 1, 2], [3, 4, 5], [6, 7]]

BF16 = mybir.dt.bfloat16
F32 = mybir.dt.float32
FP8 = mybir.dt.float8e4
DR = mybir.MatmulPerfMode.DoubleRow

# Rotating engine assignment for the bf16->fp8 z cast: DVE also feeds the
# muls + drain adds, ACT is otherwise nearly idle, Pool (gpsimd) runs copies
# at 0.6 efficiency. Per 16 pairs: 8 ACT, 5 Pool, 3 DVE.
_CAST_DVE = {0, 6, 11}
_CAST_POOL = {2, 5, 9, 13, 15}

_cache: dict = {}


def _build_program():
    if "nc" in _cache:
        return _cache["nc"]
    from contextlib import ExitStack

    nc = bacc.Bacc("TRN2", target_bir_lowering=False, debug=False)

    xt_d = nc.dram_tensor("xt", [128, 4, BLOC], BF16, kind="ExternalInput")
    gb_d = nc.dram_tensor("gb", [G, 128, BLOC], BF16, kind="ExternalInput")
    w_d = nc.dram_tensor("w", [NPAIR, 128, 2, OUT_W], FP8, kind="ExternalInput")
    bias_d = nc.dram_tensor("bias", [BLOC, OUT_W], F32, kind="ExternalInput")
    out_d = nc.dram_tensor("out", [BLOC, OUT_W], F32, kind="ExternalOutput")

    with tile.TileContext(nc) as tc, ExitStack() as ctx:
        res = ctx.enter_context(tc.tile_pool(name="res", bufs=1))
        gp = ctx.enter_context(tc.tile_pool(name="gp", bufs=12))
        wp = ctx.enter_context(tc.tile_pool(name="wp", bufs=6))
        zbp = ctx.enter_context(tc.tile_pool(name="zbp", bufs=6))
        zp = ctx.enter_context(tc.tile_pool(name="zp", bufs=8))
        op = ctx.enter_context(tc.tile_pool(name="op", bufs=3))
        bp = ctx.enter_context(tc.tile_pool(name="bp", bufs=4))
        pp = ctx.enter_context(tc.tile_pool(name="pp", bufs=1, space="PSUM"))

        # x^T resident as one [128, 4, BLOC] tile (AP for a K-tile pair must
        # come from a single tensor); loaded in 4 slices so pair 0 can start
        # after the first two.
        xt4 = res.tile([128, 4, BLOC], BF16, name="xt4", tag="xt4")
        for ib in range(4):
            nc.sync.dma_start(xt4[:, ib, :], xt_d[:, ib, :])

        # Per-sweep gate column slices [128, mw], queue alternating by group
        # parity so neither HWDGE queue carries the full gate traffic.
        gbs: dict = {}

        def issue_gb(s, gi):
            mcs = SWEEPS[s]
            m0 = mcs[0] * 128
            mw = len(mcs) * 128
            t = gp.tile([128, mw], BF16, name=f"gb{s}_{gi}", tag="gbs")
            q = nc.sync if gi % 2 == 0 else nc.gpsimd
            q.dma_start(t[:], gb_d[gi][:, m0:m0 + mw])
            gbs[(s, gi)] = t

        for gi0 in range(3):
            issue_gb(0, gi0)

        carry_z: dict = {}

        def gen_z(s, pr, mcs):
            gi = pr // 2
            xb0 = (pr % 2) * 2
            m0 = mcs[0] * 128
            mw = len(mcs) * 128
            zb = zbp.tile([128, 2, mw], BF16, name=f"zb{s}_{pr}", tag="zb")
            gsl = gbs[(s, gi)][:].unsqueeze(1).broadcast_to([128, 2, mw])
            nc.vector.tensor_mul(zb[:], xt4[:, xb0:xb0 + 2, m0:m0 + mw], gsl)
            zt = zp.tile([128, 2, mw], FP8, name=f"zt{s}_{pr}", tag="zt")
            m = pr % 16
            if m in _CAST_DVE:
                nc.vector.tensor_copy(zt[:], zb[:])
            elif m in _CAST_POOL:
                nc.gpsimd.tensor_copy(zt[:], zb[:])
            else:
                nc.scalar.activation(zt[:], zb[:],
                                     mybir.ActivationFunctionType.Copy)
            return zt

        for s, mcs in enumerate(SWEEPS):
            # ppi gets a second PSUM bank slot so the next sweep never waits
            # on this sweep's pi drain.
            ppi = pp.tile([128, 16 * len(mcs)], F32, name=f"ppi{s}",
                          tag="ppi", bufs=2)
            pmu, psg, bt = {}, {}, {}
            for j, mc in enumerate(mcs):
                pmu[mc] = pp.tile([128, CD], F32, name=f"pmu{s}_{j}",
                                  tag="pmu", bufs=3)
                psg[mc] = pp.tile([128, CD], F32, name=f"psg{s}_{j}",
                                  tag="psg", bufs=3)

            for pr in range(NPAIR):
                gi = pr // 2
                if pr % 2 == 0 and gi + 3 < G:
                    issue_gb(s, gi + 3)
                if pr == 8:
                    # Bias tiles for this sweep's drain, paced mid-sweep so
                    # they can't crowd the startup-critical loads.
                    for j2, mc2 in enumerate(mcs):
                        bt[mc2] = bp.tile([128, OUT_W], F32,
                                          name=f"bt{s}_{j2}", tag="bt")
                        nc.sync.dma_start(
                            bt[mc2][:],
                            bias_d[mc2 * 128:(mc2 + 1) * 128, :])
                wt = wp.tile([128, 2, OUT_W], FP8, name=f"wt{s}_{pr}",
                             tag="wt")
                wq = nc.sync if pr % 2 == 0 else nc.gpsimd
                wq.dma_start(wt[:], w_d[pr])
                zt = carry_z.pop((s, pr), None)
                if zt is None:
                    zt = gen_z(s, pr, mcs)
                first = pr == 0
                last = pr == NPAIR - 1
                if last:
                    # Final pair: run the sigma matmuls first so psg is
                    # ready earliest — its drain (add→Exp→Ln) is the long
                    # serial chain at the end of the sweep.
                    for j, mc in enumerate(mcs):
                        lhs = zt[:, :, j * 128:(j + 1) * 128]
                        nc.tensor.matmul(psg[mc][:], lhs, wt[:, :, C + CD:],
                                         start=False, stop=True,
                                         perf_mode=DR)
                    for j, mc in enumerate(mcs):
                        lhs = zt[:, :, j * 128:(j + 1) * 128]
                        nc.tensor.matmul(pmu[mc][:], lhs, wt[:, :, C:C + CD],
                                         start=False, stop=True,
                                         perf_mode=DR)
                        nc.tensor.matmul(ppi[:, j * 16:(j + 1) * 16], lhs,
                                         wt[:, :, 0:C], start=False,
                                         stop=True, perf_mode=DR,
                                         skip_group_check=True)
                    continue
                for j, mc in enumerate(mcs):
                    lhs = zt[:, :, j * 128:(j + 1) * 128]
                    nc.tensor.matmul(pmu[mc][:], lhs, wt[:, :, C:C + CD],
                                     start=first, stop=False, perf_mode=DR)
                    nc.tensor.matmul(psg[mc][:], lhs, wt[:, :, C + CD:],
                                     start=first, stop=False, perf_mode=DR)
                    # start=True marks the whole bank pending-zero, so only
                    # the first matmul into the shared pi bank sets it.
                    nc.tensor.matmul(ppi[:, j * 16:(j + 1) * 16], lhs,
                                     wt[:, :, 0:C], start=(first and j == 0),
                                     stop=False, perf_mode=DR,
                                     skip_group_check=True)

            # Queue the next sweep's first z-tiles (and their gate slices)
            # ahead of the drain work so PE can restart immediately at the
            # sweep boundary.
            if s + 1 < len(SWEEPS):
                for gi0 in range(3):
                    issue_gb(s + 1, gi0)
                for pr in range(2):
                    carry_z[(s + 1, pr)] = gen_z(s + 1, pr, SWEEPS[s + 1])

            # Drain, phase-batched so ACT runs exp,exp,..,ln,ln.
            # softplus(v) = ln(exp(v) + 1); the reference's +1e-7 is dropped
            # (5e-7 relative effect, far below fp8 noise).
            ots, ets = {}, {}
            for j, mc in enumerate(mcs):
                # ei-add first feeds the ACT serial chain; mu-add right
                # after frees pmu for the next sweep's opening matmul.
                ei = op.tile([128, CD], F32, name=f"ei{s}_{j}", tag="ei",
                             bufs=3)
                nc.vector.tensor_add(ei[:], psg[mc][:], bt[mc][:, C + CD:])
                ot = op.tile([128, OUT_W], F32, name=f"ot{s}_{j}", tag="ot")
                nc.vector.tensor_add(ot[:, C:C + CD], pmu[mc][:],
                                     bt[mc][:, C:C + CD])
                ots[mc] = ot
                ets[mc] = ei
            for j, mc in enumerate(mcs):
                et = op.tile([128, CD], F32, name=f"et{s}_{j}", tag="et",
                             bufs=3)
                nc.scalar.activation(et[:], ets[mc][:],
                                     mybir.ActivationFunctionType.Exp)
                ets[mc] = et
            for j, mc in enumerate(mcs):
                ot = ots[mc]
                nc.vector.tensor_add(ot[:, 0:C], ppi[:, j * 16:(j + 1) * 16],
                                     bt[mc][:, 0:C])
                nc.gpsimd.dma_start(out_d[mc * 128:(mc + 1) * 128, 0:C + CD],
                                    ot[:, 0:C + CD])
            for j, mc in enumerate(mcs):
                ot = ots[mc]
                nc.scalar.activation(ot[:, C + CD:], ets[mc][:],
                                     mybir.ActivationFunctionType.Ln,
                                     bias=1.0)
                nc.gpsimd.dma_start(out_d[mc * 128:(mc + 1) * 128, C + CD:],
                                    ot[:, C + CD:])

    nc.compile()
    _cache["nc"] = nc
    return nc


def _prep_shared(W_mu, b_mu, W_sigma, b_sigma, W_pi, b_pi):
    fp8 = ml_dtypes.float8_e4m3
    # Column order matches the reference output: [logits | loc | scale].
    w_cat = np.concatenate([W_pi, W_mu, W_sigma], axis=-1)      # [G, I, 1040]
    # K-tile pairs for DoubleRow: [pair, partition, 2, out] where
    # w_np[pr, p, i, :] = W row k = (2*pr+i)*128 + p.
    w_np = np.ascontiguousarray(
        w_cat.reshape(NPAIR, 2, 128, OUT_W).transpose(0, 2, 1, 3)
        .astype(fp8))
    b_cat = np.concatenate([b_pi, b_mu, b_sigma],
                           axis=-1).astype(np.float32)          # [G, 1040]
    return w_np, b_cat


def _core_inputs(x, g, w_np, b_cat, c):
    bf16 = ml_dtypes.bfloat16
    xs = x[c * BLOC:(c + 1) * BLOC]
    gs = g[c * BLOC:(c + 1) * BLOC]
    # x^T packed as [partition, i-block, sample]: xt4[p, ib, b] = x[b, ib*128+p]
    xT = np.ascontiguousarray(
        xs.T.astype(bf16).reshape(4, 128, BLOC).transpose(1, 0, 2))
    gT = gs.T.astype(bf16)                                      # [32, 1024]
    gb = np.ascontiguousarray(
        np.broadcast_to(gT[:, None, :], (G, 128, BLOC)))        # [32,128,1024]
    bias = np.ascontiguousarray(gs.astype(np.float32) @ b_cat)  # [1024, 1040]
    return {"xt": xT, "gb": gb, "w": w_np, "bias": bias}


def kernel(x, g, W_mu, b_mu, W_sigma, b_sigma, W_pi, b_pi):
    nc = _build_program()
    w_np, b_cat = _prep_shared(W_mu, b_mu, W_sigma, b_sigma, W_pi, b_pi)
    in_maps = [_core_inputs(x, g, w_np, b_cat, c) for c in range(NCORES)]
    res = run_bass_kernel_spmd(nc, in_maps, core_ids=list(range(NCORES)))
    out = np.concatenate(
        [res.results[c]["out"] for c in range(NCORES)], axis=0)
    return np.ascontiguousarray(out.astype(np.float32))


# revision 7
# speedup vs baseline: 2.7541x; 1.4148x over previous
"""GroupGMM Trainium2 kernel (fp8 DoubleRow, resident weights).

Computes, for B=8192 samples with soft group-mixture weights over G=32 groups:
    logits = einsum("bi,gio,bg->bo", x, W_pi, g) + g @ b_pi        [B, 16]
    loc    = einsum(... W_mu ...)   + g @ b_mu                     [B, 512]
    scale  = softplus(einsum(... W_sigma ...) + g @ b_sigma)+1e-7  [B, 512]
    out    = concat([logits, loc, scale], -1)                      [B, 1040]

Strategy: data-parallel over batch across 8 NeuronCores (1024 rows each).
The group einsum folds into one matmul with contraction K = G*I = 16384 via
z[b,(g,i)] = g[b,g] * x[b,i], run in fp8e4 (e4m3) with the PE's DoubleRow
perf mode (two 128-row K-tiles per pass at 0.5 cycles/row; measured overall
rel err ~1e-2 vs the 2e-2 gate). At fp8-DR speed the kernel sits on the
cost model's ridge: PE needs ~217ns per K-pair-chunk, the single 360GB/s
DMA_ENGINES resource needs ~100us for the minimal HBM traffic, and every
DMA dispatch costs ~0.7us of its sequencer (HWDGE) or ~1us of the Pool
engine (SWDGE). Hence:
  - The 17MB fp8 weight tensor is RESIDENT: loaded once during sweep 1 in
    9 large DMAs (re-streaming per sweep would triple W traffic and pin
    every sweep at ~60us of DMA).
  - Gates load as per-sweep [128, 8-groups, mw] column-slice tiles (4 DMAs
    per sweep), x^T as one [128, 4, BLOC] tile in 2 DMAs; all loads ride
    the sync HWDGE queue; output writes ride the ACT HWDGE queue so a
    compute-dependent store can never head-of-line-block a load.
  - z tiles are built per K-pair: ONE DVE multiply [128, 2, mw] bf16 (x^T
    pair slice x gate row broadcast by a stride-0 AP — 2x DVE mode), then
    cast bf16->fp8 on a rotating engine (8 ACT / 6 Pool / 2 DVE-direct per
    16) because a DVE multiply cannot write fp8 at 2x and no single helper
    engine can match the PE pace.
Per 128-sample chunk each pair issues 3 DR matmuls (mu 512, sigma 512,
pi 16 cols; lhsT [128, 2, 128]) — the exact shapes concourse's tile_matmul
emits. PSUM: 3 chunks x (mu+sg) + 2 pi banks = 8, so the batch runs in 3
sweeps ([0..2],[3..5],[6..7]). The bias term g @ b_cat is precomputed on
the host in f32 and added at drain time on DVE; sigma gets softplus via
ACT Exp+Ln.
"""

import numpy as np
import ml_dtypes

import concourse.bass as bass
import concourse.tile as tile
from concourse import bacc, mybir
from concourse.bass_utils import run_bass_kernel_spmd

B, I, G, C, D = 8192, 512, 32, 16, 32
CD = C * D                      # 512
OUT_W = C + 2 * CD              # 1040
NCORES = 8
BLOC = B // NCORES              # 1024
KTOT = G * I                    # 16384
NKT = KTOT // 128               # 128 K-tiles
NPAIR = NKT // 2                # 64 DoubleRow K-tile pairs
NMC = BLOC // 128               # 8 sample chunks per core
SWEEPS = [[0, 1, 2], [3, 4, 5], [6, 7]]
# Resident W loads: first two small so the opening matmuls start early.
WCHUNKS = [2, 6, 8, 8, 8, 8, 8, 8, 8]

BF16 = mybir.dt.bfloat16
F32 = mybir.dt.float32
FP8 = mybir.dt.float8e4
DR = mybir.MatmulPerfMode.DoubleRow

# Rotating engine for the bf16->fp8 z cast (by pair index mod 16).
_CAST_DVE = {5, 13}
_CAST_POOL = {1, 3, 7, 9, 11, 15}

_cache: dict = {}


def _build_program():
    if "nc" in _cache:
        return _cache["nc"]
    from contextlib import ExitStack

    nc = bacc.Bacc("TRN2", target_bir_lowering=False, debug=False)

    xt_d = nc.dram_tensor("xt", [128, 4, BLOC], BF16, kind="ExternalInput")
    gb_d = nc.dram_tensor("gb", [128, G, BLOC], BF16, kind="ExternalInput")
    w_d = nc.dram_tensor("w", [NPAIR, 128, 2, OUT_W], FP8, kind="ExternalInput")
    bias_d = nc.dram_tensor("bias", [128, NMC, OUT_W], F32,
                            kind="ExternalInput")
    out_d = nc.dram_tensor("out", [128, NMC, OUT_W], F32,
                           kind="ExternalOutput")

    with tile.TileContext(nc) as tc, ExitStack() as ctx:
        res = ctx.enter_context(tc.tile_pool(name="res", bufs=1))
        gp = ctx.enter_context(tc.tile_pool(name="gp", bufs=3))
        zbp = ctx.enter_context(tc.tile_pool(name="zbp", bufs=4))
        zp = ctx.enter_context(tc.tile_pool(name="zp", bufs=6))
        op = ctx.enter_context(tc.tile_pool(name="op", bufs=3))
        bp = ctx.enter_context(tc.tile_pool(name="bp", bufs=3))
        pp = ctx.enter_context(tc.tile_pool(name="pp", bufs=1, space="PSUM"))

        # ---- resident loads (sync HWDGE queue, in issue order) ----
        # x^T packed [128, 4, BLOC]; first half early so pair 0's z-mul can
        # start, gates for groups 0-7, then the first W pairs.
        xt4 = res.tile([128, 4, BLOC], BF16, name="xt4", tag="xt4")
        nc.sync.dma_start(xt4[:, 0:2, :], xt_d[:, 0:2, :])

        # Per-sweep gate slices: [128, 8 groups, mw] tiles, 4 per sweep.
        gbt: dict = {}

        def issue_gb(s, t):
            mcs = SWEEPS[s]
            m0 = mcs[0] * 128
            mw = len(mcs) * 128
            tl = gp.tile([128, 8, mw], BF16, name=f"gb{s}_{t}", tag="gbs")
            nc.sync.dma_start(tl[:], gb_d[:, t * 8:(t + 1) * 8, m0:m0 + mw])
            gbt[(s, t)] = tl

        issue_gb(0, 0)

        # Resident W: 64 individual [128, 2, OUT_W] tiles. A DoubleRow
        # matmul's moving operand must come from a tile whose partition
        # stride equals the slice's free size — slicing one big 4D tile
        # produces silently wrong results (BIRSim), so per-pair tiles it is.
        # The 64 HWDGE dispatches pipeline under the ~47us of W transfer.
        wres = []
        for pr in range(NPAIR):
            t = res.tile([128, 2, OUT_W], FP8, name=f"w{pr}", tag=f"w{pr}")
            wres.append(t)

        def wslice(pr, c0, c1):
            return wres[pr][:, :, c0:c1]

        nc.sync.dma_start(wres[0][:], w_d[0])
        nc.sync.dma_start(wres[1][:], w_d[1])
        nc.sync.dma_start(xt4[:, 2:4, :], xt_d[:, 2:4, :])
        for pr in range(2, 8):
            nc.sync.dma_start(wres[pr][:], w_d[pr])
        issue_gb(0, 1)

        carry_z: dict = {}

        def gen_z(s, pr, mcs):
            gi = pr // 2
            xb0 = (pr % 2) * 2
            m0 = mcs[0] * 128
            mw = len(mcs) * 128
            gsl = gbt[(s, gi // 8)][:, gi % 8, :].unsqueeze(1).broadcast_to(
                [128, 2, mw])
            xsl = xt4[:, xb0:xb0 + 2, m0:m0 + mw]
            zt = zp.tile([128, 2, mw], FP8, name=f"zt{s}_{pr}", tag="zt")
            m = pr % 16
            if m in _CAST_DVE:
                # Direct fp8-out multiply (1x DVE) — cheaper than mul+copy
                # when DVE does both anyway.
                nc.vector.tensor_mul(zt[:], xsl, gsl)
                return zt
            zb = zbp.tile([128, 2, mw], BF16, name=f"zb{s}_{pr}", tag="zb")
            nc.vector.tensor_mul(zb[:], xsl, gsl)
            if m in _CAST_POOL:
                nc.gpsimd.tensor_copy(zt[:], zb[:])
            else:
                nc.scalar.activation(zt[:], zb[:],
                                     mybir.ActivationFunctionType.Copy)
            return zt

        for s, mcs in enumerate(SWEEPS):
            # ppi gets a second PSUM bank slot so the next sweep never waits
            # on this sweep's pi drain.
            ppi = pp.tile([128, 16 * len(mcs)], F32, name=f"ppi{s}",
                          tag="ppi", bufs=2)
            pmu, psg, bt = {}, {}, {}
            for j, mc in enumerate(mcs):
                pmu[mc] = pp.tile([128, CD], F32, name=f"pmu{s}_{j}",
                                  tag="pmu", bufs=3)
                psg[mc] = pp.tile([128, CD], F32, name=f"psg{s}_{j}",
                                  tag="psg", bufs=3)

            for pr in range(NPAIR):
                if s == 0:
                    # Interleave the remaining resident W loads and this
                    # sweep's gate tiles into the W-paced stream, keeping
                    # the HWDGE queue ~8 pairs ahead of the PE.
                    if pr + 8 < NPAIR:
                        nc.sync.dma_start(wres[pr + 8][:], w_d[pr + 8])
                    if pr == 8:
                        issue_gb(0, 2)
                    elif pr == 24:
                        issue_gb(0, 3)
                else:
                    if pr == 2:
                        issue_gb(s, 1)
                    elif pr == 18:
                        issue_gb(s, 2)
                    elif pr == 34:
                        issue_gb(s, 3)
                if pr == 12:
                    for j2, mc2 in enumerate(mcs):
                        bt[mc2] = bp.tile([128, OUT_W], F32,
                                          name=f"bt{s}_{j2}", tag="bt")
                        nc.sync.dma_start(bt[mc2][:], bias_d[:, mc2, :])
                zt = carry_z.pop((s, pr), None)
                if zt is None:
                    zt = gen_z(s, pr, mcs)
                first = pr == 0
                last = pr == NPAIR - 1
                if last:
                    # Final pair: sigma matmuls first so psg is ready
                    # earliest — its drain (add→Exp→Ln) is the long serial
                    # chain at the end of the sweep.
                    for j, mc in enumerate(mcs):
                        lhs = zt[:, :, j * 128:(j + 1) * 128]
                        nc.tensor.matmul(psg[mc][:], lhs,
                                         wslice(pr, C + CD, OUT_W),
                                         start=False, stop=True, perf_mode=DR)
                    for j, mc in enumerate(mcs):
                        lhs = zt[:, :, j * 128:(j + 1) * 128]
                        nc.tensor.matmul(pmu[mc][:], lhs,
                                         wslice(pr, C, C + CD),
                                         start=False, stop=True, perf_mode=DR)
                        nc.tensor.matmul(ppi[:, j * 16:(j + 1) * 16], lhs,
                                         wslice(pr, 0, C), start=False,
                                         stop=True, perf_mode=DR,
                                         skip_group_check=True)
                    continue
                for j, mc in enumerate(mcs):
                    lhs = zt[:, :, j * 128:(j + 1) * 128]
                    nc.tensor.matmul(pmu[mc][:], lhs, wslice(pr, C, C + CD),
                                     start=first, stop=False, perf_mode=DR)
                    nc.tensor.matmul(psg[mc][:], lhs,
                                     wslice(pr, C + CD, OUT_W),
                                     start=first, stop=False, perf_mode=DR)
                    # start=True marks the whole bank pending-zero, so only
                    # the first matmul into the shared pi bank sets it.
                    nc.tensor.matmul(ppi[:, j * 16:(j + 1) * 16], lhs,
                                     wslice(pr, 0, C),
                                     start=(first and j == 0), stop=False,
                                     perf_mode=DR, skip_group_check=True)

            # Queue the next sweep's first z-tiles (and gate slices) ahead
            # of the drain work so PE restarts immediately at the boundary.
            if s + 1 < len(SWEEPS):
                issue_gb(s + 1, 0)
                for pr in range(2):
                    carry_z[(s + 1, pr)] = gen_z(s + 1, pr, SWEEPS[s + 1])

            # Drain, phase-batched so ACT runs exp,exp,..,ln,ln.
            # softplus(v) = ln(exp(v) + 1); the reference's +1e-7 is dropped
            # (5e-7 relative effect, far below fp8 noise).
            ots, ets = {}, {}
            for j, mc in enumerate(mcs):
                # ei-add first feeds the ACT serial chain; mu-add right
                # after frees pmu for the next sweep's opening matmul.
                ei = op.tile([128, CD], F32, name=f"ei{s}_{j}", tag="ei",
                             bufs=3)
                nc.vector.tensor_add(ei[:], psg[mc][:], bt[mc][:, C + CD:])
                ot = op.tile([128, OUT_W], F32, name=f"ot{s}_{j}", tag="ot")
                nc.vector.tensor_add(ot[:, C:C + CD], pmu[mc][:],
                                     bt[mc][:, C:C + CD])
                ots[mc] = ot
                ets[mc] = ei
            for j, mc in enumerate(mcs):
                et = op.tile([128, CD], F32, name=f"et{s}_{j}", tag="et",
                             bufs=2)
                nc.scalar.activation(et[:], ets[mc][:],
                                     mybir.ActivationFunctionType.Exp)
                ets[mc] = et
            for j, mc in enumerate(mcs):
                ot = ots[mc]
                nc.vector.tensor_add(ot[:, 0:C], ppi[:, j * 16:(j + 1) * 16],
                                     bt[mc][:, 0:C])
                # Output writes ride the ACT HWDGE queue: they depend on
                # compute, and on the load queue they would head-of-line
                # block the next sweep's gate/bias tiles.
                nc.scalar.dma_start(out_d[:, mc, 0:C + CD], ot[:, 0:C + CD])
            for j, mc in enumerate(mcs):
                ot = ots[mc]
                nc.scalar.activation(ot[:, C + CD:], ets[mc][:],
                                     mybir.ActivationFunctionType.Ln,
                                     bias=1.0)
                nc.scalar.dma_start(out_d[:, mc, C + CD:], ot[:, C + CD:])

    nc.compile()
    _cache["nc"] = nc
    return nc


def _prep_shared(W_mu, b_mu, W_sigma, b_sigma, W_pi, b_pi):
    fp8 = ml_dtypes.float8_e4m3
    # Column order matches the reference output: [logits | loc | scale].
    w_cat = np.concatenate([W_pi, W_mu, W_sigma], axis=-1)      # [G, I, 1040]
    # K-tile pairs for DoubleRow: [pair, partition, 2, out] where
    # w_np[pr, p, i, :] = W row k = (2*pr+i)*128 + p.
    w_np = np.ascontiguousarray(
        w_cat.reshape(NPAIR, 2, 128, OUT_W).transpose(0, 2, 1, 3)
        .astype(fp8))
    b_cat = np.concatenate([b_pi, b_mu, b_sigma],
                           axis=-1).astype(np.float32)          # [G, 1040]
    return w_np, b_cat


def _core_inputs(x, g, w_np, b_cat, c):
    bf16 = ml_dtypes.bfloat16
    xs = x[c * BLOC:(c + 1) * BLOC]
    gs = g[c * BLOC:(c + 1) * BLOC]
    # x^T packed [partition, i-block, sample]: xt[p, ib, b] = x[b, ib*128+p]
    xT = np.ascontiguousarray(
        xs.T.astype(bf16).reshape(4, 128, BLOC).transpose(1, 0, 2))
    # gates broadcast across partitions: gb[p, g, b] = g[b, g]
    gT = gs.T.astype(bf16)                                      # [32, 1024]
    gb = np.ascontiguousarray(
        np.broadcast_to(gT[None, :, :], (128, G, BLOC)))        # [128,32,1024]
    # bias packed [partition, chunk, out]: bias[p, mc, o] = (g @ b)[mc*128+p, o]
    bias = np.ascontiguousarray(
        (gs.astype(np.float32) @ b_cat).reshape(NMC, 128, OUT_W)
        .transpose(1, 0, 2))
    return {"xt": xT, "gb": gb, "w": w_np, "bias": bias}


def kernel(x, g, W_mu, b_mu, W_sigma, b_sigma, W_pi, b_pi):
    nc = _build_program()
    w_np, b_cat = _prep_shared(W_mu, b_mu, W_sigma, b_sigma, W_pi, b_pi)
    in_maps = [_core_inputs(x, g, w_np, b_cat, c) for c in range(NCORES)]
    res = run_bass_kernel_spmd(nc, in_maps, core_ids=list(range(NCORES)))
    out = np.concatenate(
        [res.results[c]["out"].transpose(1, 0, 2).reshape(BLOC, OUT_W)
         for c in range(NCORES)], axis=0)
    return np.ascontiguousarray(out.astype(np.float32))
